# revision 1
# baseline (speedup 1.0000x reference)
"""Trainium2 Bass kernel for bidirectional masked-LSTM + attention pooling + FC head.

Problem (hardcoded shapes): B=64, T=512, E=256, H=512, OH=1024.
  - x [B,T,E] f32, lengths [B] i32, attn_w [T] f32
  - per-direction LSTM weights Wih [4H,E], Whh [4H,H], biases [4H]
  - fc1 [OH,2H]+[OH], fc2 [T,OH]+[T]
  - out: logits [B,T] f32, padded positions = -1e30

Sharding: 8 cores = 4 batch groups (16 seqs) x 2 directions. Each core runs one
direction's full 512-step recurrence for its 16 sequences. Attention pooling is
folded into the recurrence as a masked weighted accumulate (per-(t,b) scale
table precomputed on host, which also implements sequence reversal masking for
the backward direction). The FC head runs on every core; forward/backward
pooled partials are combined with a pairwise AllReduce.

Layouts (per core):
  h "hidden-tiled" [128, K_CH*16]: h[b, hid] at partition hid%128, col (hid//128)*16+b.
  gates PSUM tiled [128, m*16+b] per gate-chunk m (gate g=m*128+p), gate order
  permuted to [i, f, o, g] so i,f share one sigmoid and g is one tanh.
"""

import os

import numpy as np

import concourse.bass as bass
import concourse.tile as tile
from concourse import bacc, mybir
from concourse.bass_utils import run_bass_kernel_spmd

B, T, E, H, OH = 64, 512, 256, 512, 1024
G = 4 * H          # 2048 gates
BL = 16            # batch per core
M_CH = G // 128    # 16 gate chunks
K_CH = H // 128    # 4 hidden chunks
E_CH = E // 128    # 2 input chunks
MO_CH = OH // 128  # 8
MT_CH = T // 128   # 4
NBLK = 32          # xp prefetch block (timesteps)

f32 = mybir.dt.float32
f16 = mybir.dt.float16
AF = mybir.ActivationFunctionType
ALU = mybir.AluOpType

# gate permutation: torch order [i,f,g,o] -> kernel order [i,f,o,g]
# perm[new_pos] = old_index  (applied to rows of Wih/Whh and bias)
_GPERM = np.concatenate([
    np.arange(0, H),          # i
    np.arange(H, 2 * H),      # f
    np.arange(3 * H, 4 * H),  # o
    np.arange(2 * H, 3 * H),  # g
])


def _bc_free(ap, reps, width):
    """AP that broadcasts a [P, width] slice to [P, reps, width] via stride-0."""
    return bass.AP(
        tensor=ap.tensor,
        offset=ap.offset,
        ap=[ap.ap[0], [0, reps]] + list(ap.ap[1:]),
    )


def build_nc(t_steps=T, use_collective=True):
    nc = bacc.Bacc("TRN2", target_bir_lowering=False, num_devices=8)

    # ---- DRAM parameters (per-core payloads prepared on host) ----
    xt = nc.declare_dram_parameter("xt", [E_CH, 128, BL * T], f16, isOutput=False)
    wih = nc.declare_dram_parameter("wih", [E_CH, 128, G], f16, isOutput=False)
    whh = nc.declare_dram_parameter("whh", [K_CH, 128, G], f16, isOutput=False)
    biasT = nc.declare_dram_parameter("biasT", [128, M_CH], f32, isOutput=False)
    sc = nc.declare_dram_parameter("sc", [128, T, BL], f16, isOutput=False)
    w1t = nc.declare_dram_parameter("w1t", [K_CH, 128, OH], f16, isOutput=False)
    b1T = nc.declare_dram_parameter("b1T", [128, MO_CH], f32, isOutput=False)
    w2t = nc.declare_dram_parameter("w2t", [MO_CH, 128, T], f16, isOutput=False)
    b2T = nc.declare_dram_parameter("b2T", [128, MT_CH], f32, isOutput=False)

    out_logits = nc.declare_dram_parameter("out_logits", [128, MT_CH * BL], f32,
                                           isOutput=True)
    out_pooled = nc.declare_dram_parameter("out_pooled", [128, K_CH * BL], f32,
                                           isOutput=True)

    xp_dram = nc.dram_tensor("xp_scratch", [M_CH, BL, 128, T], f16)
    ar_in = nc.dram_tensor("ar_in", [128, MO_CH * BL], f32)
    ar_out = nc.dram_tensor("ar_out", [128, MO_CH * BL], f32)

    with tile.TileContext(nc) as tc:
        with tc.tile_pool(name="const", bufs=1) as const_pool:
            whh_sb = const_pool.tile([128, K_CH, G], f16)
            for k in range(K_CH):
                nc.sync.dma_start(out=whh_sb[:, k, :], in_=whh[k])
            biasT_sb = const_pool.tile([128, M_CH], f32)
            nc.sync.dma_start(out=biasT_sb, in_=biasT[:, :])
            sc_sb = const_pool.tile([128, T, BL], f16)
            nc.sync.dma_start(out=sc_sb, in_=sc[:, :, :])
            w1t_sb = const_pool.tile([128, K_CH, OH], f16)
            for k in range(K_CH):
                nc.sync.dma_start(out=w1t_sb[:, k, :], in_=w1t[k])
            b1T_sb = const_pool.tile([128, MO_CH], f32)
            nc.sync.dma_start(out=b1T_sb, in_=b1T[:, :])
            w2t_sb = const_pool.tile([128, MO_CH, T], f16)
            for k in range(MO_CH):
                nc.sync.dma_start(out=w2t_sb[:, k, :], in_=w2t[k])
            b2T_sb = const_pool.tile([128, MT_CH], f32)
            nc.sync.dma_start(out=b2T_sb, in_=b2T[:, :])

            # ---- Phase 1: input projection xp = x @ Wih_perm.T + bias ----
            with tc.tile_pool(name="proj_in", bufs=1) as proj_in, \
                 tc.tile_pool(name="proj_ps", bufs=4, space="PSUM") as proj_ps, \
                 tc.tile_pool(name="proj_ev", bufs=4) as proj_ev:
                wih_sb = proj_in.tile([128, E_CH, G], f16)
                for k in range(E_CH):
                    nc.sync.dma_start(out=wih_sb[:, k, :], in_=wih[k])
                xt_sb = proj_in.tile([128, E_CH, BL * T], f16)
                for k in range(E_CH):
                    nc.sync.dma_start(out=xt_sb[:, k, :], in_=xt[k])

                for m in range(M_CH):
                    for b in range(BL):
                        ps = proj_ps.tile([128, 512], f32, tag="pp")
                        for k in range(E_CH):
                            nc.tensor.matmul(
                                ps,
                                lhsT=wih_sb[:, k, m * 128:(m + 1) * 128],
                                rhs=xt_sb[:, k, b * T:b * T + 512],
                                start=(k == 0),
                                stop=(k == E_CH - 1),
                            )
                        xp_sb = proj_ev.tile([128, 512], f16, tag="xps")
                        if (m * BL + b) % 2 == 0:
                            nc.vector.tensor_scalar(
                                out=xp_sb, in0=ps,
                                scalar1=biasT_sb[:, m:m + 1], scalar2=None,
                                op0=ALU.add,
                            )
                        else:
                            nc.scalar.activation(
                                out=xp_sb, in_=ps, func=AF.Identity,
                                bias=biasT_sb[:, m:m + 1], scale=1.0,
                            )
                        nc.sync.dma_start(out=xp_dram[m, b], in_=xp_sb)

            # ---- Phase 2: recurrence ----
            with tc.tile_pool(name="state", bufs=1) as state_pool:
                h_sb = state_pool.tile([128, K_CH * BL], f16)
                c_sb = state_pool.tile([128, K_CH * BL], f32)
                acc = state_pool.tile([128, K_CH * BL], f32)
                nc.vector.memset(h_sb, 0.0)
                nc.vector.memset(c_sb, 0.0)
                nc.gpsimd.memset(acc, 0.0)

                with tc.tile_pool(name="xpb", bufs=2) as xpb_pool, \
                     tc.tile_pool(name="rec_ps", bufs=2, space="PSUM") as rec_ps, \
                     tc.tile_pool(name="work", bufs=2) as work:
                    n_blocks = (t_steps + NBLK - 1) // NBLK
                    for blk in range(n_blocks):
                        t0 = blk * NBLK
                        nt = min(NBLK, t_steps - t0)
                        xpb = xpb_pool.tile([128, M_CH * BL, NBLK], f16, tag="xpb")
                        for m in range(M_CH):
                            for b in range(BL):
                                nc.sync.dma_start(
                                    out=xpb[:, m * BL + b, :nt],
                                    in_=xp_dram[m, b, :, t0:t0 + nt],
                                )
                        for tt in range(nt):
                            t = t0 + tt
                            # PE: group order g, i+f, o
                            psg = rec_ps.tile([128, 64], f32, tag="psg")
                            pif = rec_ps.tile([128, 128], f32, tag="psif")
                            pso = rec_ps.tile([128, 64], f32, tag="pso")
                            for j, m in enumerate(range(12, 16)):  # g
                                for k in range(K_CH):
                                    nc.tensor.matmul(
                                        psg[:, j * 16:(j + 1) * 16],
                                        lhsT=whh_sb[:, k, m * 128:(m + 1) * 128],
                                        rhs=h_sb[:, k * BL:(k + 1) * BL],
                                        start=(k == 0), stop=(k == K_CH - 1),
                                    )
                            gsA = work.tile([128, 64], f32, tag="gsA")
                            nc.vector.tensor_add(
                                out=gsA, in0=psg,
                                in1=xpb[:, 192:256, tt],
                            )
                            tg = work.tile([128, 64], f32, tag="tg")
                            nc.scalar.activation(out=tg, in_=gsA, func=AF.Tanh)

                            for j, m in enumerate(range(0, 8)):  # i, f
                                for k in range(K_CH):
                                    nc.tensor.matmul(
                                        pif[:, j * 16:(j + 1) * 16],
                                        lhsT=whh_sb[:, k, m * 128:(m + 1) * 128],
                                        rhs=h_sb[:, k * BL:(k + 1) * BL],
                                        start=(k == 0), stop=(k == K_CH - 1),
                                    )
                            sifA = work.tile([128, 128], f32, tag="sifA")
                            nc.vector.tensor_add(
                                out=sifA, in0=pif, in1=xpb[:, 0:128, tt])
                            sif = work.tile([128, 128], f32, tag="sif")
                            nc.scalar.activation(out=sif, in_=sifA, func=AF.Sigmoid)

                            for j, m in enumerate(range(8, 12)):  # o
                                for k in range(K_CH):
                                    nc.tensor.matmul(
                                        pso[:, j * 16:(j + 1) * 16],
                                        lhsT=whh_sb[:, k, m * 128:(m + 1) * 128],
                                        rhs=h_sb[:, k * BL:(k + 1) * BL],
                                        start=(k == 0), stop=(k == K_CH - 1),
                                    )
                            soA = work.tile([128, 64], f32, tag="soA")
                            nc.vector.tensor_add(
                                out=soA, in0=pso, in1=xpb[:, 128:192, tt])
                            so = work.tile([128, 64], f32, tag="so")
                            nc.scalar.activation(out=so, in_=soA, func=AF.Sigmoid)

                            t1 = work.tile([128, 64], f32, tag="t1")
                            nc.vector.tensor_mul(out=t1, in0=sif[:, 0:64], in1=tg)
                            t2 = work.tile([128, 64], f32, tag="t2")
                            nc.vector.tensor_mul(out=t2, in0=sif[:, 64:128], in1=c_sb)
                            nc.vector.tensor_add(out=c_sb, in0=t1, in1=t2)
                            tch = work.tile([128, 64], f32, tag="tch")
                            nc.scalar.activation(out=tch, in_=c_sb, func=AF.Tanh)
                            nc.vector.tensor_mul(out=h_sb, in0=so, in1=tch)

                            pt = work.tile([128, 64], f32, tag="pt")
                            nc.gpsimd.tensor_mul(
                                out=pt, in0=h_sb,
                                in1=_bc_free(sc_sb[:, t, :], K_CH, BL),
                            )
                            nc.gpsimd.tensor_add(out=acc, in0=acc, in1=pt)

                # ---- Phase 3: head ----
                with tc.tile_pool(name="head", bufs=1) as head, \
                     tc.tile_pool(name="head_ps", bufs=1, space="PSUM") as head_ps:
                    nc.sync.dma_start(out=out_pooled[:, :], in_=acc)
                    acch = head.tile([128, K_CH * BL], f16)
                    nc.vector.tensor_copy(out=acch, in_=acc)
                    ps1 = head_ps.tile([128, MO_CH * BL], f32)
                    for mo in range(MO_CH):
                        for k in range(K_CH):
                            nc.tensor.matmul(
                                ps1[:, mo * BL:(mo + 1) * BL],
                                lhsT=w1t_sb[:, k, mo * 128:(mo + 1) * 128],
                                rhs=acch[:, k * BL:(k + 1) * BL],
                                start=(k == 0), stop=(k == K_CH - 1),
                            )
                    p1_sb = head.tile([128, MO_CH * BL], f32)
                    nc.vector.tensor_copy(out=p1_sb, in_=ps1)
                    if use_collective:
                        nc.sync.dma_start(out=ar_in[:, :], in_=p1_sb)
                        nc.gpsimd.collective_compute(
                            "AllReduce",
                            ALU.add,
                            replica_groups=[[0, 1], [2, 3], [4, 5], [6, 7]],
                            ins=[ar_in[:, :].opt()],
                            outs=[ar_out[:, :].opt()],
                        )
                        r_sb = head.tile([128, MO_CH * BL], f32)
                        nc.sync.dma_start(out=r_sb, in_=ar_out[:, :])
                    else:
                        r_sb = p1_sb
                    h1 = head.tile([128, MO_CH * BL], f16)
                    for mo in range(MO_CH):
                        nc.scalar.activation(
                            out=h1[:, mo * BL:(mo + 1) * BL],
                            in_=r_sb[:, mo * BL:(mo + 1) * BL],
                            func=AF.Relu,
                            bias=b1T_sb[:, mo:mo + 1],
                        )
                    ps2 = head_ps.tile([128, MT_CH * BL], f32)
                    for mt in range(MT_CH):
                        for ko in range(MO_CH):
                            nc.tensor.matmul(
                                ps2[:, mt * BL:(mt + 1) * BL],
                                lhsT=w2t_sb[:, ko, mt * 128:(mt + 1) * 128],
                                rhs=h1[:, ko * BL:(ko + 1) * BL],
                                start=(ko == 0), stop=(ko == MO_CH - 1),
                            )
                    lg_sb = head.tile([128, MT_CH * BL], f32)
                    for mt in range(MT_CH):
                        nc.vector.tensor_scalar(
                            out=lg_sb[:, mt * BL:(mt + 1) * BL],
                            in0=ps2[:, mt * BL:(mt + 1) * BL],
                            scalar1=b2T_sb[:, mt:mt + 1], scalar2=None,
                            op0=ALU.add,
                        )
                    nc.sync.dma_start(out=out_logits[:, :], in_=lg_sb)

    nc.compile()
    return nc


def _tile_kxg(w, n_k):
    """[G, K] weight (already permuted rows) -> [n_k, 128, G] fp16 with
    out[k, kk, g] = w[g, k*128+kk]."""
    K = n_k * 128
    wt = w.T.astype(np.float32)  # [K, G]
    return np.ascontiguousarray(
        wt.reshape(n_k, 128, -1)).astype(np.float16)


def prep_core_inputs(x_dir, wih_p, whh_p, bias_p, sc_tb, fc1_w, fc1_b,
                     fc2_w, fc2_b, direction):
    """Build the per-core input map. x_dir [BL, T, E] f32 (already reversed for
    bwd), weights already gate-permuted."""
    ins = {}
    # xt [E_CH, 128, BL*T]: xt[k][kk][b*T+t] = x_dir[b,t,k*128+kk]
    xtt = x_dir.transpose(2, 0, 1).reshape(E_CH, 128, BL * T)
    ins["xt"] = np.ascontiguousarray(xtt).astype(np.float16)
    ins["wih"] = _tile_kxg(wih_p, E_CH)
    ins["whh"] = _tile_kxg(whh_p, K_CH)
    ins["biasT"] = np.ascontiguousarray(
        bias_p.reshape(M_CH, 128).T).astype(np.float32)
    # sc [128, T, BL] replicated over partitions
    ins["sc"] = np.broadcast_to(
        sc_tb.astype(np.float16)[None, :, :], (128, T, BL)).copy()
    w1d = fc1_w[:, direction * H:(direction + 1) * H]  # [OH, H]
    ins["w1t"] = _tile_kxg(w1d, K_CH)
    ins["b1T"] = np.ascontiguousarray(
        fc1_b.reshape(MO_CH, 128).T).astype(np.float32)
    ins["w2t"] = _tile_kxg(fc2_w, MO_CH)
    ins["b2T"] = np.ascontiguousarray(
        fc2_b.reshape(MT_CH, 128).T).astype(np.float32)
    return ins


_NC_CACHE = {}
LAST_RESULT = None


def kernel(x, lengths, attn_w, Wih_f, Whh_f, bih_f, bhh_f,
           Wih_b, Whh_b, bih_b, bhh_b, fc1_w, fc1_b, fc2_w, fc2_b):
    x = np.asarray(x, np.float32)
    lengths = np.asarray(lengths, np.int32)
    attn_w = np.asarray(attn_w, np.float32)
    use_collective = os.environ.get("LSTM_NO_COLLECTIVE", "0") != "1"

    key = (T, use_collective)
    if key not in _NC_CACHE:
        _NC_CACHE[key] = build_nc(T, use_collective)
    nc = _NC_CACHE[key]

    # softmax over attn_w (host glue, exact fp32 as in reference)
    aw = attn_w - attn_w.max()
    e = np.exp(aw)
    scores = (e / e.sum()).astype(np.float32)  # [T]

    tr = np.arange(T)
    # forward sc: sc_f[t, b] = scores[t] * (t < len_b)
    # backward sc: sc_b[tau, b] = scores[len_b-1-tau] * (tau < len_b)
    in_maps = []
    for g in range(4):
        bsl = slice(g * BL, (g + 1) * BL)
        xg = x[bsl]                      # [BL, T, E]
        lg = lengths[bsl]                # [BL]
        mask = tr[:, None] < lg[None, :]  # [T, BL]
        sc_f = scores[:, None] * mask
        idx = np.clip(lg[None, :] - 1 - tr[:, None], 0, T - 1)  # [T, BL]
        sc_b = scores[idx] * mask
        # x reversed per sequence (zeros past length)
        idxc = np.clip(lg[:, None] - 1 - tr[None, :], 0, T - 1)  # [BL, T]
        xrev = np.take_along_axis(xg, idxc[:, :, None], axis=1)
        xrev = xrev * mask.T[:, :, None]

        bias_f = (bih_f + bhh_f)[_GPERM].astype(np.float32)
        bias_b = (bih_b + bhh_b)[_GPERM].astype(np.float32)
        in_maps.append(prep_core_inputs(
            xg, Wih_f[_GPERM], Whh_f[_GPERM], bias_f, sc_f,
            fc1_w, fc1_b, fc2_w, fc2_b, 0))
        in_maps.append(prep_core_inputs(
            xrev, Wih_b[_GPERM], Whh_b[_GPERM], bias_b, sc_b,
            fc1_w, fc1_b, fc2_w, fc2_b, 1))

    trace = os.environ.get("LSTM_TRACE", "0") == "1"
    res = run_bass_kernel_spmd(nc, in_maps, list(range(8)), trace=trace)
    results = res.results
    global LAST_RESULT
    LAST_RESULT = res

    out = np.empty((B, T), np.float32)
    for g in range(4):
        if use_collective:
            lt = results[2 * g]["out_logits"]  # [128, MT_CH*BL]
            lg_out = lt.reshape(128, MT_CH, BL).transpose(2, 1, 0).reshape(BL, T)
        else:
            # host head from pooled partials
            pf = results[2 * g]["out_pooled"]
            pb = results[2 * g + 1]["out_pooled"]
            pooled = np.concatenate(
                [pf.reshape(128, K_CH, BL).transpose(2, 1, 0).reshape(BL, H),
                 pb.reshape(128, K_CH, BL).transpose(2, 1, 0).reshape(BL, H)],
                axis=1)
            h1 = np.maximum(pooled @ fc1_w.T + fc1_b, 0.0)
            lg_out = h1 @ fc2_w.T + fc2_b
        out[g * BL:(g + 1) * BL] = lg_out
    tmask = tr[None, :] < lengths[:, None]
    return np.where(tmask, out, np.float32(-1e30)).astype(np.float32)



# revision 4
# speedup vs baseline: 1.5200x; 1.5200x over previous
"""Trainium2 Bass kernel for bidirectional masked-LSTM + attention pooling + FC head.

Problem (hardcoded shapes): B=64, T=512, E=256, H=512, OH=1024.
  - x [B,T,E] f32, lengths [B] i32, attn_w [T] f32
  - per-direction LSTM weights Wih [4H,E], Whh [4H,H], biases [4H]
  - fc1 [OH,2H]+[OH], fc2 [T,OH]+[T]
  - out: logits [B,T] f32, padded positions = -1e30

Sharding: 8 cores = 4 batch groups (16 seqs) x 2 directions. Each core runs one
direction's full 512-step recurrence for its 16 sequences. Attention pooling is
folded into the recurrence as a masked weighted accumulate (per-(t,b) scale
table precomputed on host, which also implements sequence reversal masking for
the backward direction). The FC head runs on every core; forward/backward
pooled partials are combined with a pairwise AllReduce.

Layouts (per core):
  h "hidden-tiled" [128, K_CH*16]: h[b, hid] at partition hid%128, col (hid//128)*16+b.
  gates PSUM tiled [128, m*16+b] per gate-chunk m (gate g=m*128+p), gate order
  permuted to [i, f, o, g] so i,f share one sigmoid and g is one tanh.

Per-step schedule (latency-oriented): the xp addend is folded into PSUM via an
identity matmul emitted at the head of each step (runs during the previous
step's elementwise tail), then gate groups run i,f -> g -> o so the
sigmoid(i,f), f*c, tanh(g), i*g, and c-update all overlap the remaining matmul
groups; only sigmoid(o) -> tanh(c) -> h stays exposed after the last matmul.
"""

import os

import numpy as np

import concourse.bass as bass
import concourse.tile as tile
from concourse import bacc, mybir
from concourse.bass_utils import run_bass_kernel_spmd

B, T, E, H, OH = 64, 512, 256, 512, 1024
G = 4 * H          # 2048 gates
BL = 16            # batch per core
M_CH = G // 128    # 16 gate chunks
K_CH = H // 128    # 4 hidden chunks
E_CH = E // 128    # 2 input chunks
MO_CH = OH // 128  # 8
MT_CH = T // 128   # 4
NBLK = 32          # xp prefetch block (timesteps)

f32 = mybir.dt.float32
f16 = mybir.dt.float16
AF = mybir.ActivationFunctionType
ALU = mybir.AluOpType

# gate permutation: torch order [i,f,g,o] -> kernel order [i,f,o,g]
# perm[new_pos] = old_index  (applied to rows of Wih/Whh and bias)
_GPERM = np.concatenate([
    np.arange(0, H),          # i
    np.arange(H, 2 * H),      # f
    np.arange(3 * H, 4 * H),  # o
    np.arange(2 * H, 3 * H),  # g
])


def _bc_free(ap, reps, width):
    """AP that broadcasts a [P, width] slice to [P, reps, width] via stride-0."""
    return bass.AP(
        tensor=ap.tensor,
        offset=ap.offset,
        ap=[ap.ap[0], [0, reps]] + list(ap.ap[1:]),
    )


def build_nc(t_steps=T, use_collective=True):
    nc = bacc.Bacc("TRN2", target_bir_lowering=False, num_devices=8)

    # ---- DRAM parameters (per-core payloads prepared on host) ----
    xt = nc.declare_dram_parameter("xt", [E_CH, 128, BL * T], f16, isOutput=False)
    wih = nc.declare_dram_parameter("wih", [E_CH, 128, G], f16, isOutput=False)
    whh = nc.declare_dram_parameter("whh", [K_CH, 128, G], f16, isOutput=False)
    biasT = nc.declare_dram_parameter("biasT", [128, M_CH], f32, isOutput=False)
    sc = nc.declare_dram_parameter("sc", [128, T, BL], f16, isOutput=False)
    ident = nc.declare_dram_parameter("ident", [128, 128], f16, isOutput=False)
    w1t = nc.declare_dram_parameter("w1t", [K_CH, 128, OH], f16, isOutput=False)
    b1T = nc.declare_dram_parameter("b1T", [128, MO_CH], f32, isOutput=False)
    w2t = nc.declare_dram_parameter("w2t", [MO_CH, 128, T], f16, isOutput=False)
    b2T = nc.declare_dram_parameter("b2T", [128, MT_CH], f32, isOutput=False)

    out_logits = nc.declare_dram_parameter("out_logits", [128, MT_CH * BL], f32,
                                           isOutput=True)
    out_pooled = nc.declare_dram_parameter("out_pooled", [128, K_CH * BL], f32,
                                           isOutput=True)

    xp_dram = nc.dram_tensor("xp_scratch", [M_CH, 128, BL, T], f16)
    ar_in = nc.dram_tensor("ar_in", [128, MO_CH * BL], f32)
    ar_out = nc.dram_tensor("ar_out", [128, MO_CH * BL], f32)

    with tile.TileContext(nc) as tc:
        with tc.tile_pool(name="const", bufs=1) as const_pool:
            whh_sb = const_pool.tile([128, K_CH, G], f16)
            for k in range(K_CH):
                nc.sync.dma_start(out=whh_sb[:, k, :], in_=whh[k])
            biasT_sb = const_pool.tile([128, M_CH], f32)
            nc.sync.dma_start(out=biasT_sb, in_=biasT[:, :])
            sc_sb = const_pool.tile([128, T, BL], f16)
            nc.sync.dma_start(out=sc_sb, in_=sc[:, :, :])
            ident_sb = const_pool.tile([128, 128], f16)
            nc.sync.dma_start(out=ident_sb, in_=ident[:, :])
            w1t_sb = const_pool.tile([128, K_CH, OH], f16)
            for k in range(K_CH):
                nc.sync.dma_start(out=w1t_sb[:, k, :], in_=w1t[k])
            b1T_sb = const_pool.tile([128, MO_CH], f32)
            nc.sync.dma_start(out=b1T_sb, in_=b1T[:, :])
            w2t_sb = const_pool.tile([128, MO_CH, T], f16)
            for k in range(MO_CH):
                nc.sync.dma_start(out=w2t_sb[:, k, :], in_=w2t[k])
            b2T_sb = const_pool.tile([128, MT_CH], f32)
            nc.sync.dma_start(out=b2T_sb, in_=b2T[:, :])

            # ---- Phase 1: input projection xp = x @ Wih_perm.T + bias ----
            with tc.tile_pool(name="proj_in", bufs=1) as proj_in, \
                 tc.tile_pool(name="proj_ps", bufs=4, space="PSUM") as proj_ps, \
                 tc.tile_pool(name="proj_ev", bufs=4) as proj_ev:
                wih_sb = proj_in.tile([128, E_CH, G], f16)
                for k in range(E_CH):
                    nc.sync.dma_start(out=wih_sb[:, k, :], in_=wih[k])
                xt_sb = proj_in.tile([128, E_CH, BL * T], f16)
                for k in range(E_CH):
                    nc.sync.dma_start(out=xt_sb[:, k, :], in_=xt[k])

                for m in range(M_CH):
                    for b in range(BL):
                        ps = proj_ps.tile([128, 512], f32, tag="pp")
                        for k in range(E_CH):
                            nc.tensor.matmul(
                                ps,
                                lhsT=wih_sb[:, k, m * 128:(m + 1) * 128],
                                rhs=xt_sb[:, k, b * T:b * T + 512],
                                start=(k == 0),
                                stop=(k == E_CH - 1),
                            )
                        xp_sb = proj_ev.tile([128, 512], f16, tag="xps")
                        if (m * BL + b) % 2 == 0:
                            nc.vector.tensor_scalar(
                                out=xp_sb, in0=ps,
                                scalar1=biasT_sb[:, m:m + 1], scalar2=None,
                                op0=ALU.add,
                            )
                        else:
                            nc.scalar.activation(
                                out=xp_sb, in_=ps, func=AF.Identity,
                                bias=biasT_sb[:, m:m + 1], scale=1.0,
                            )
                        nc.sync.dma_start(out=xp_dram[m, :, b, :], in_=xp_sb)

            # ---- Phase 2: recurrence ----
            with tc.tile_pool(name="state", bufs=1) as state_pool:
                h_sb = state_pool.tile([128, K_CH * BL], f16)
                c_sb = state_pool.tile([128, K_CH * BL], f32)
                acc = state_pool.tile([128, K_CH * BL], f32)
                nc.vector.memset(h_sb, 0.0)
                nc.vector.memset(c_sb, 0.0)
                nc.gpsimd.memset(acc, 0.0)

                with tc.tile_pool(name="xpb", bufs=2) as xpb_pool, \
                     tc.tile_pool(name="rec_ps", bufs=2, space="PSUM") as rec_ps, \
                     tc.tile_pool(name="work", bufs=2) as work:
                    n_blocks = (t_steps + NBLK - 1) // NBLK
                    for blk in range(n_blocks):
                        t0 = blk * NBLK
                        nt = min(NBLK, t_steps - t0)
                        xpb = xpb_pool.tile([128, M_CH * BL, NBLK], f16, tag="xpb")
                        for m in range(M_CH):
                            nc.sync.dma_start(
                                out=xpb[:, m * BL:(m + 1) * BL, :nt],
                                in_=xp_dram[m, :, :, t0:t0 + nt],
                            )
                        for tt in range(nt):
                            t = t0 + tt
                            pif = rec_ps.tile([128, 128], f32, tag="pif")
                            psg = rec_ps.tile([128, 64], f32, tag="psg")
                            pso = rec_ps.tile([128, 64], f32, tag="pso")
                            # xp folds: no h dependency, so the PE can run
                            # these during the previous step's tail.
                            nc.tensor.matmul(pif, lhsT=ident_sb,
                                             rhs=xpb[:, 0:128, tt],
                                             start=True, stop=False)
                            nc.tensor.matmul(psg, lhsT=ident_sb,
                                             rhs=xpb[:, 192:256, tt],
                                             start=True, stop=False)
                            nc.tensor.matmul(pso, lhsT=ident_sb,
                                             rhs=xpb[:, 128:192, tt],
                                             start=True, stop=False)
                            # i,f group (m=0..7)
                            for m in range(8):
                                for k in range(K_CH):
                                    nc.tensor.matmul(
                                        pif[:, m * 16:(m + 1) * 16],
                                        lhsT=whh_sb[:, k, m * 128:(m + 1) * 128],
                                        rhs=h_sb[:, k * BL:(k + 1) * BL],
                                        start=False, stop=(k == K_CH - 1),
                                    )
                            sif = work.tile([128, 128], f32, tag="sif")
                            nc.scalar.activation(out=sif, in_=pif, func=AF.Sigmoid)
                            t2 = work.tile([128, 64], f32, tag="t2")
                            nc.vector.tensor_mul(out=t2, in0=sif[:, 64:128],
                                                 in1=c_sb)
                            # g group (m=12..15)
                            for j, m in enumerate(range(12, 16)):
                                for k in range(K_CH):
                                    nc.tensor.matmul(
                                        psg[:, j * 16:(j + 1) * 16],
                                        lhsT=whh_sb[:, k, m * 128:(m + 1) * 128],
                                        rhs=h_sb[:, k * BL:(k + 1) * BL],
                                        start=False, stop=(k == K_CH - 1),
                                    )
                            tg = work.tile([128, 64], f32, tag="tg")
                            nc.scalar.activation(out=tg, in_=psg, func=AF.Tanh)
                            t1 = work.tile([128, 64], f32, tag="t1")
                            nc.vector.tensor_mul(out=t1, in0=sif[:, 0:64], in1=tg)
                            nc.vector.tensor_add(out=c_sb, in0=t1, in1=t2)
                            # o group (m=8..11)
                            for j, m in enumerate(range(8, 12)):
                                for k in range(K_CH):
                                    nc.tensor.matmul(
                                        pso[:, j * 16:(j + 1) * 16],
                                        lhsT=whh_sb[:, k, m * 128:(m + 1) * 128],
                                        rhs=h_sb[:, k * BL:(k + 1) * BL],
                                        start=False, stop=(k == K_CH - 1),
                                    )
                            so = work.tile([128, 64], f32, tag="so")
                            nc.scalar.activation(out=so, in_=pso, func=AF.Sigmoid)
                            tch = work.tile([128, 64], f32, tag="tch")
                            nc.scalar.activation(out=tch, in_=c_sb, func=AF.Tanh)
                            nc.vector.tensor_mul(out=h_sb, in0=so, in1=tch)

                            pt = work.tile([128, 64], f32, tag="pt")
                            nc.gpsimd.tensor_mul(
                                out=pt, in0=h_sb,
                                in1=_bc_free(sc_sb[:, t, :], K_CH, BL),
                            )
                            nc.gpsimd.tensor_add(out=acc, in0=acc, in1=pt)

                # ---- Phase 3: head ----
                with tc.tile_pool(name="head", bufs=1) as head, \
                     tc.tile_pool(name="head_ps", bufs=1, space="PSUM") as head_ps:
                    nc.sync.dma_start(out=out_pooled[:, :], in_=acc)
                    acch = head.tile([128, K_CH * BL], f16)
                    nc.vector.tensor_copy(out=acch, in_=acc)
                    ps1 = head_ps.tile([128, MO_CH * BL], f32)
                    for mo in range(MO_CH):
                        for k in range(K_CH):
                            nc.tensor.matmul(
                                ps1[:, mo * BL:(mo + 1) * BL],
                                lhsT=w1t_sb[:, k, mo * 128:(mo + 1) * 128],
                                rhs=acch[:, k * BL:(k + 1) * BL],
                                start=(k == 0), stop=(k == K_CH - 1),
                            )
                    p1_sb = head.tile([128, MO_CH * BL], f32)
                    nc.vector.tensor_copy(out=p1_sb, in_=ps1)
                    if use_collective:
                        nc.sync.dma_start(out=ar_in[:, :], in_=p1_sb)
                        nc.gpsimd.collective_compute(
                            "AllReduce",
                            ALU.add,
                            replica_groups=[[0, 1], [2, 3], [4, 5], [6, 7]],
                            ins=[ar_in[:, :].opt()],
                            outs=[ar_out[:, :].opt()],
                        )
                        r_sb = head.tile([128, MO_CH * BL], f32)
                        nc.sync.dma_start(out=r_sb, in_=ar_out[:, :])
                    else:
                        r_sb = p1_sb
                    h1 = head.tile([128, MO_CH * BL], f16)
                    for mo in range(MO_CH):
                        nc.scalar.activation(
                            out=h1[:, mo * BL:(mo + 1) * BL],
                            in_=r_sb[:, mo * BL:(mo + 1) * BL],
                            func=AF.Relu,
                            bias=b1T_sb[:, mo:mo + 1],
                        )
                    ps2 = head_ps.tile([128, MT_CH * BL], f32)
                    for mt in range(MT_CH):
                        for ko in range(MO_CH):
                            nc.tensor.matmul(
                                ps2[:, mt * BL:(mt + 1) * BL],
                                lhsT=w2t_sb[:, ko, mt * 128:(mt + 1) * 128],
                                rhs=h1[:, ko * BL:(ko + 1) * BL],
                                start=(ko == 0), stop=(ko == MO_CH - 1),
                            )
                    lg_sb = head.tile([128, MT_CH * BL], f32)
                    for mt in range(MT_CH):
                        nc.vector.tensor_scalar(
                            out=lg_sb[:, mt * BL:(mt + 1) * BL],
                            in0=ps2[:, mt * BL:(mt + 1) * BL],
                            scalar1=b2T_sb[:, mt:mt + 1], scalar2=None,
                            op0=ALU.add,
                        )
                    nc.sync.dma_start(out=out_logits[:, :], in_=lg_sb)

    nc.compile()
    return nc


def _tile_kxg(w, n_k):
    """[G, K] weight (already permuted rows) -> [n_k, 128, G] fp16 with
    out[k, kk, g] = w[g, k*128+kk]."""
    K = n_k * 128
    wt = w.T.astype(np.float32)  # [K, G]
    return np.ascontiguousarray(
        wt.reshape(n_k, 128, -1)).astype(np.float16)


def prep_core_inputs(x_dir, wih_p, whh_p, bias_p, sc_tb, fc1_w, fc1_b,
                     fc2_w, fc2_b, direction):
    """Build the per-core input map. x_dir [BL, T, E] f32 (already reversed for
    bwd), weights already gate-permuted."""
    ins = {}
    # xt [E_CH, 128, BL*T]: xt[k][kk][b*T+t] = x_dir[b,t,k*128+kk]
    xtt = x_dir.transpose(2, 0, 1).reshape(E_CH, 128, BL * T)
    ins["xt"] = np.ascontiguousarray(xtt).astype(np.float16)
    ins["wih"] = _tile_kxg(wih_p, E_CH)
    ins["whh"] = _tile_kxg(whh_p, K_CH)
    ins["biasT"] = np.ascontiguousarray(
        bias_p.reshape(M_CH, 128).T).astype(np.float32)
    # sc [128, T, BL] replicated over partitions
    ins["sc"] = np.broadcast_to(
        sc_tb.astype(np.float16)[None, :, :], (128, T, BL)).copy()
    ins["ident"] = np.eye(128, dtype=np.float16)
    w1d = fc1_w[:, direction * H:(direction + 1) * H]  # [OH, H]
    ins["w1t"] = _tile_kxg(w1d, K_CH)
    ins["b1T"] = np.ascontiguousarray(
        fc1_b.reshape(MO_CH, 128).T).astype(np.float32)
    ins["w2t"] = _tile_kxg(fc2_w, MO_CH)
    ins["b2T"] = np.ascontiguousarray(
        fc2_b.reshape(MT_CH, 128).T).astype(np.float32)
    return ins


_NC_CACHE = {}
LAST_RESULT = None


def kernel(x, lengths, attn_w, Wih_f, Whh_f, bih_f, bhh_f,
           Wih_b, Whh_b, bih_b, bhh_b, fc1_w, fc1_b, fc2_w, fc2_b):
    x = np.asarray(x, np.float32)
    lengths = np.asarray(lengths, np.int32)
    attn_w = np.asarray(attn_w, np.float32)
    use_collective = os.environ.get("LSTM_NO_COLLECTIVE", "0") != "1"

    key = (T, use_collective)
    if key not in _NC_CACHE:
        _NC_CACHE[key] = build_nc(T, use_collective)
    nc = _NC_CACHE[key]

    # softmax over attn_w (host glue, exact fp32 as in reference)
    aw = attn_w - attn_w.max()
    e = np.exp(aw)
    scores = (e / e.sum()).astype(np.float32)  # [T]

    tr = np.arange(T)
    # forward sc: sc_f[t, b] = scores[t] * (t < len_b)
    # backward sc: sc_b[tau, b] = scores[len_b-1-tau] * (tau < len_b)
    in_maps = []
    for g in range(4):
        bsl = slice(g * BL, (g + 1) * BL)
        xg = x[bsl]                      # [BL, T, E]
        lg = lengths[bsl]                # [BL]
        mask = tr[:, None] < lg[None, :]  # [T, BL]
        sc_f = scores[:, None] * mask
        idx = np.clip(lg[None, :] - 1 - tr[:, None], 0, T - 1)  # [T, BL]
        sc_b = scores[idx] * mask
        # x reversed per sequence (zeros past length)
        idxc = np.clip(lg[:, None] - 1 - tr[None, :], 0, T - 1)  # [BL, T]
        xrev = np.take_along_axis(xg, idxc[:, :, None], axis=1)
        xrev = xrev * mask.T[:, :, None]

        bias_f = (bih_f + bhh_f)[_GPERM].astype(np.float32)
        bias_b = (bih_b + bhh_b)[_GPERM].astype(np.float32)
        in_maps.append(prep_core_inputs(
            xg, Wih_f[_GPERM], Whh_f[_GPERM], bias_f, sc_f,
            fc1_w, fc1_b, fc2_w, fc2_b, 0))
        in_maps.append(prep_core_inputs(
            xrev, Wih_b[_GPERM], Whh_b[_GPERM], bias_b, sc_b,
            fc1_w, fc1_b, fc2_w, fc2_b, 1))

    trace = os.environ.get("LSTM_TRACE", "0") == "1"
    res = run_bass_kernel_spmd(nc, in_maps, list(range(8)), trace=trace)
    results = res.results
    global LAST_RESULT
    LAST_RESULT = res

    out = np.empty((B, T), np.float32)
    for g in range(4):
        if use_collective:
            lt = results[2 * g]["out_logits"]  # [128, MT_CH*BL]
            lg_out = lt.reshape(128, MT_CH, BL).transpose(2, 1, 0).reshape(BL, T)
        else:
            # host head from pooled partials
            pf = results[2 * g]["out_pooled"]
            pb = results[2 * g + 1]["out_pooled"]
            pooled = np.concatenate(
                [pf.reshape(128, K_CH, BL).transpose(2, 1, 0).reshape(BL, H),
                 pb.reshape(128, K_CH, BL).transpose(2, 1, 0).reshape(BL, H)],
                axis=1)
            h1 = np.maximum(pooled @ fc1_w.T + fc1_b, 0.0)
            lg_out = h1 @ fc2_w.T + fc2_b
        out[g * BL:(g + 1) * BL] = lg_out
    tmask = tr[None, :] < lengths[:, None]
    return np.where(tmask, out, np.float32(-1e30)).astype(np.float32)
